# revision 16
# baseline (speedup 1.0000x reference)
"""Causal self-attention on 8 Trainium2 NeuronCores.

Full inputs in, full output out. Sharding: core c -> (batch b = c//2,
head-group hg = c%2 covering 8 of 16 heads). Each core computes QKV
projections for its head slice, causal flash-attention in a transposed
layout (S^T = keys x queries, so softmax denominators come from a ones
column appended to V and no on-device transposes are needed), and a
partial output projection over its 512 feature columns. The host sums
the two partials per batch and adds the bias.

All host-side reshapes/transposes (x^T, weight slices) are numpy; the
device consumes them directly. Matmul operands are float16 (full PE
rate, fp32 PSUM accumulation; ~7e-4 end-to-end relative error). The
two heads of a feature block issue S^T matmuls on disjoint PE row
groups (partitions 0-63 / 64-127) so they run concurrently, and
diagonal blocks only compute the causally-live trapezoid of columns.
"""
import sys

if "/opt/trn_rl_repo" not in sys.path:
    sys.path.insert(0, "/opt/trn_rl_repo")

import numpy as np

import concourse.bass as bass
import concourse.tile as tile
from concourse import bacc, mybir
from concourse.bass_utils import run_bass_kernel_spmd

F32 = mybir.dt.float32
F16 = mybir.dt.float16
AF = mybir.ActivationFunctionType

B, T, C = 4, 2048, 1024
H, D = 16, 64
N_CORES = 8
HPC = 8            # heads per core
FPC = HPC * D      # feats per core = 512
NEG = -30000.0
QB = 512           # query block
NQB = T // QB      # 4
NKK = T // 128     # 16 key chunks
NCC = C // 128     # 8 contraction chunks
NFB = FPC // 128   # 4 feature blocks (head pairs)

_cached = {}


def _build_program():
    nc = bacc.Bacc("TRN2", target_bir_lowering=False, debug=False,
                   num_devices=N_CORES)

    xT_d = nc.dram_tensor("xT", [C, T], F16, kind="ExternalInput").ap()
    wqT_d = nc.dram_tensor("wqT", [C, FPC], F16, kind="ExternalInput").ap()
    wkT_d = nc.dram_tensor("wkT", [C, FPC], F16, kind="ExternalInput").ap()
    wvT_d = nc.dram_tensor("wvT", [C, FPC], F16, kind="ExternalInput").ap()
    wpT_d = nc.dram_tensor("wpT", [FPC, C], F16, kind="ExternalInput").ap()
    ident_d = nc.dram_tensor("ident", [128, 128], F16, kind="ExternalInput").ap()
    tri_d = nc.dram_tensor("tri", [128, 128], F16, kind="ExternalInput").ap()
    ones_d = nc.dram_tensor("ones", [1, 64], F16, kind="ExternalInput").ap()
    out_d = nc.dram_tensor("out", [T, C], F32, kind="ExternalOutput").ap()

    with tile.TileContext(nc) as tc:
        with tc.tile_pool(name="persist", bufs=1) as persist:
            qt_sb = persist.tile([128, NFB, T], F16, tag="qt")
            kt_sb = persist.tile([128, NFB, T], F16, tag="kt")
            v_sb = persist.tile([128, NKK, HPC, D + 1], F16, tag="v")
            ident = persist.tile([128, 128], F16, tag="ident")
            tri = persist.tile([128, 128], F16, tag="tri")
            ones_col = persist.tile([1, 64], F16, tag="ones")
            nc.sync.dma_start(ident, ident_d)
            nc.sync.dma_start(tri, tri_d)
            nc.sync.dma_start(ones_col, ones_d)
            # ones column of V' (PV matmul then emits softmax denominators)
            nc.gpsimd.memset(v_sb[:, :, :, D:D + 1], 1.0)

            # ---------------- phase 1: QKV projections ----------------
            with tc.tile_pool(name="wqkv", bufs=1) as wqkv, \
                 tc.tile_pool(name="xt", bufs=2) as xtp, \
                 tc.tile_pool(name="ps1", bufs=4, space="PSUM") as ps1:
                wq_sb = wqkv.tile([128, NCC, FPC], F16, tag="wq")
                wk_sb = wqkv.tile([128, NCC, FPC], F16, tag="wk")
                wv_sb = wqkv.tile([128, NCC, FPC], F16, tag="wv")
                wq_r = wqT_d.rearrange("(c p) f -> p c f", p=128)
                wk_r = wkT_d.rearrange("(c p) f -> p c f", p=128)
                wv_r = wvT_d.rearrange("(c p) f -> p c f", p=128)
                xT_r = xT_d.rearrange("(c p) t -> p c t", p=128)

                # first token block's x lands before the weight bulk so the
                # first matmuls can start early
                xt0 = xtp.tile([128, NCC, QB], F16, tag="xt", name="xt0")
                for cc in range(NCC):
                    nc.sync.dma_start(xt0[:, cc, :], xT_r[:, cc, 0:QB])
                for cc in range(NCC):
                    nc.sync.dma_start(wq_sb[:, cc, :], wq_r[:, cc, :])
                    nc.sync.dma_start(wk_sb[:, cc, :], wk_r[:, cc, :])
                    nc.sync.dma_start(wv_sb[:, cc, :], wv_r[:, cc, :])

                # warm the PE clock gate while input DMAs stream in
                warm = ps1.tile([128, 512], F32, tag="ps1", name="warm")
                for i in range(60):
                    nc.tensor.matmul(warm[:, 0:128], ident, ident,
                                     start=True, stop=True,
                                     skip_group_check=True)

                for tb in range(NQB):
                    if tb == 0:
                        xt = xt0
                    else:
                        xt = xtp.tile([128, NCC, QB], F16, tag="xt",
                                      name="xt")
                        for cc in range(NCC):
                            nc.sync.dma_start(
                                xt[:, cc, :],
                                xT_r[:, cc, tb * QB:(tb + 1) * QB])
                    for fb in range(NFB):
                        qps = ps1.tile([128, QB], F32, tag="ps1", name="qps")
                        for cc in range(NCC):
                            nc.tensor.matmul(
                                qps, wq_sb[:, cc, fb * 128:(fb + 1) * 128],
                                xt[:, cc, :],
                                start=(cc == 0), stop=(cc == NCC - 1))
                        nc.vector.tensor_copy(
                            qt_sb[:, fb, tb * QB:(tb + 1) * QB], qps)
                        kps = ps1.tile([128, QB], F32, tag="ps1", name="kps")
                        for cc in range(NCC):
                            nc.tensor.matmul(
                                kps, wk_sb[:, cc, fb * 128:(fb + 1) * 128],
                                xt[:, cc, :],
                                start=(cc == 0), stop=(cc == NCC - 1))
                        nc.vector.tensor_copy(
                            kt_sb[:, fb, tb * QB:(tb + 1) * QB], kps)
                    for tt in range(4):
                        vps = ps1.tile([128, FPC], F32, tag="ps1", name="vps")
                        for cc in range(NCC):
                            nc.tensor.matmul(
                                vps, xt[:, cc, tt * 128:(tt + 1) * 128],
                                wv_sb[:, cc, :],
                                start=(cc == 0), stop=(cc == NCC - 1))
                        nc.vector.tensor_copy(
                            v_sb[:, tb * 4 + tt, :, 0:D],
                            vps.rearrange("p (h d) -> p h d", h=HPC))

            # ------------- phase 2: attention + projection -------------
            with tc.tile_pool(name="const2", bufs=1) as const2, \
                 tc.tile_pool(name="pt", bufs=4) as ptp, \
                 tc.tile_pool(name="yt", bufs=2) as ytp, \
                 tc.tile_pool(name="rcp", bufs=2) as rcpp, \
                 tc.tile_pool(name="sums", bufs=2) as sumsp, \
                 tc.tile_pool(name="outsb", bufs=3) as outp, \
                 tc.tile_pool(name="st", bufs=2, space="PSUM") as stp, \
                 tc.tile_pool(name="pv", bufs=1, space="PSUM") as pvp, \
                 tc.tile_pool(name="bc", bufs=1, space="PSUM") as bcp, \
                 tc.tile_pool(name="prj", bufs=1, space="PSUM") as prjp:
                wp_sb = const2.tile([128, NFB, C], F16, tag="wp")
                nc.sync.dma_start(wp_sb, wpT_d.rearrange("(c p) f -> p c f", p=128))

                for qb in range(NQB):
                    yt = ytp.tile([128, NFB, QB], F16, tag="yt", name="yt")
                    for fb in range(NFB):
                        pv = [pvp.tile([65, QB], F32, tag=f"pv{h2}",
                                       name=f"pv{h2}")
                              for h2 in range(2)]
                        nkk = 4 * qb + 4
                        for kk in range(nkk):
                            dl = kk - 4 * qb
                            j0 = 128 * dl if dl >= 0 else 0
                            st = stp.tile([128, 2, QB], F32, tag="st",
                                          name="st")
                            for h2 in range(2):
                                p0, p1 = 64 * h2, 64 * h2 + 64
                                nc.tensor.matmul(
                                    st[:, h2, j0:QB],
                                    kt_sb[p0:p1, fb, kk * 128:(kk + 1) * 128],
                                    qt_sb[p0:p1, fb, qb * QB + j0:(qb + 1) * QB],
                                    start=True, stop=True,
                                    skip_group_check=True)
                            ptile = ptp.tile([128, 2, QB], F16, tag="pt",
                                             name="ptile")
                            nc.scalar.activation(
                                ptile[:, :, j0:QB], st[:, :, j0:QB], AF.Exp)
                            if dl >= 0:
                                # zero the causally-dead triangle of the
                                # diagonal band on the vector engine (both
                                # heads in one strided op; the 0-step middle
                                # dim re-reads the same mask tile)
                                band = ptile[:, :, j0:j0 + 128]
                                nc.vector.tensor_mul(
                                    band, band,
                                    bass.AP(tri.tensor, tri.offset,
                                            [tri.ap[0], [0, 2], tri.ap[1]]))
                            for h2 in range(2):
                                h = 2 * fb + h2
                                nc.tensor.matmul(
                                    pv[h2][:, j0:QB], v_sb[:, kk, h, :],
                                    ptile[:, h2, j0:QB],
                                    start=(kk == 0), stop=(kk == nkk - 1),
                                    skip_group_check=True)
                        for h2 in range(2):
                            p0, p1 = 64 * h2, 64 * h2 + 64
                            # evacuate the accumulator to SBUF on the mostly
                            # idle scalar engine so the PSUM bank frees up
                            # for the next feature block immediately
                            pvs = sumsp.tile([64, QB], F16, tag="pvs",
                                             name="pvs")
                            nc.scalar.copy(pvs, pv[h2][0:D, :])
                            sums = sumsp.tile([1, QB], F16, tag="sums",
                                              name="sums")
                            nc.scalar.copy(sums, pv[h2][D:D + 1, :])
                            bc = bcp.tile([64, QB], F32, tag="bc", name="bc")
                            nc.tensor.matmul(bc, ones_col, sums,
                                             start=True, stop=True)
                            rcp = rcpp.tile([64, QB], F32, tag="rcp",
                                            name="rcp")
                            nc.vector.reciprocal_approx_fast(out=rcp, in_=bc)
                            nc.vector.tensor_mul(yt[p0:p1, fb, :],
                                                 pvs, rcp)
                    for tt in range(4):
                        osb = outp.tile([128, C], F32, tag="osb", name="osb")
                        for ofc in range(2):
                            prj = prjp.tile([128, 512], F32, tag="prj",
                                            name="prj")
                            for cc in range(NFB):
                                nc.tensor.matmul(
                                    prj,
                                    yt[:, cc, tt * 128:(tt + 1) * 128],
                                    wp_sb[:, cc, ofc * 512:(ofc + 1) * 512],
                                    start=(cc == 0), stop=(cc == NFB - 1))
                            nc.vector.tensor_copy(
                                osb[:, ofc * 512:(ofc + 1) * 512], prj)
                        nc.sync.dma_start(
                            out_d[qb * QB + tt * 128:qb * QB + (tt + 1) * 128, :],
                            osb)

    nc.compile()
    return nc


def _host_inputs(x, Wk, Wq, Wv, Wp):
    """Build the 8 per-core input maps (host-side slicing/transposes)."""
    ident_np = np.eye(128, dtype=np.float16)
    p = np.arange(128)[:, None]
    jj = np.arange(128)[None, :]
    tri_np = np.where(jj < p, 0.0, 1.0).astype(np.float16)
    ones_np = np.ones((1, 64), dtype=np.float16)

    in_maps = []
    for c in range(N_CORES):
        b, hg = c // 2, c % 2
        fs = slice(hg * FPC, (hg + 1) * FPC)
        in_maps.append({
            "xT": np.ascontiguousarray(x[b].T).astype(np.float16),
            "wqT": np.ascontiguousarray((Wq[fs, :] * 0.125).T).astype(np.float16),
            "wkT": np.ascontiguousarray(Wk[fs, :].T).astype(np.float16),
            "wvT": np.ascontiguousarray(Wv[fs, :].T).astype(np.float16),
            "wpT": np.ascontiguousarray(Wp[:, fs].T).astype(np.float16),
            "ident": ident_np,
            "tri": tri_np,
            "ones": ones_np,
        })
    return in_maps


def kernel(x, Wk, Wq, Wv, Wp, bp, _trace=False):
    x = np.asarray(x, dtype=np.float32)
    Wk = np.asarray(Wk, dtype=np.float32)
    Wq = np.asarray(Wq, dtype=np.float32)
    Wv = np.asarray(Wv, dtype=np.float32)
    Wp = np.asarray(Wp, dtype=np.float32)
    bp = np.asarray(bp, dtype=np.float32)

    if "nc" not in _cached:
        _cached["nc"] = _build_program()
    nc = _cached["nc"]

    in_maps = _host_inputs(x, Wk, Wq, Wv, Wp)
    res = run_bass_kernel_spmd(nc, in_maps, core_ids=list(range(N_CORES)),
                               trace=_trace)
    _cached["last_result"] = res

    out = np.empty((B, T, C), dtype=np.float32)
    for b in range(B):
        out[b] = (res.results[2 * b]["out"].astype(np.float32)
                  + res.results[2 * b + 1]["out"]
                  + bp[None, :])
    return out


# revision 17
# speedup vs baseline: 1.0944x; 1.0944x over previous
"""Causal self-attention on 8 Trainium2 NeuronCores.

Full inputs in, full output out. Sharding: core c -> (batch b = c//2,
head-group hg = c%2 covering 8 of 16 heads). Each core computes QKV
projections for its head slice, causal flash-attention in a transposed
layout (S^T = keys x queries, so softmax denominators come from a ones
column appended to V and no on-device transposes are needed), and a
partial output projection over its 512 feature columns. The host sums
the two partials per batch and adds the bias.

All host-side reshapes/transposes (x^T, weight slices) are numpy; the
device consumes them directly. Matmul operands are float16 (full PE
rate, fp32 PSUM accumulation; ~7e-4 end-to-end relative error). The
two heads of a feature block issue S^T matmuls on disjoint PE row
groups (partitions 0-63 / 64-127) so they run concurrently, and
diagonal blocks only compute the causally-live trapezoid of columns.
"""
import sys

if "/opt/trn_rl_repo" not in sys.path:
    sys.path.insert(0, "/opt/trn_rl_repo")

import numpy as np

import concourse.bass as bass
import concourse.tile as tile
from concourse import bacc, mybir
from concourse.bass_utils import run_bass_kernel_spmd

F32 = mybir.dt.float32
F16 = mybir.dt.float16
AF = mybir.ActivationFunctionType

B, T, C = 4, 2048, 1024
H, D = 16, 64
N_CORES = 8
HPC = 8            # heads per core
FPC = HPC * D      # feats per core = 512
NEG = -30000.0
QB = 512           # query block
NQB = T // QB      # 4
NKK = T // 128     # 16 key chunks
NCC = C // 128     # 8 contraction chunks
NFB = FPC // 128   # 4 feature blocks (head pairs)

_cached = {}


def _build_program():
    nc = bacc.Bacc("TRN2", target_bir_lowering=False, debug=False,
                   num_devices=N_CORES)

    xT_d = nc.dram_tensor("xT", [C, T], F16, kind="ExternalInput").ap()
    wqT_d = nc.dram_tensor("wqT", [C, FPC], F16, kind="ExternalInput").ap()
    wkT_d = nc.dram_tensor("wkT", [C, FPC], F16, kind="ExternalInput").ap()
    wvT_d = nc.dram_tensor("wvT", [C, FPC], F16, kind="ExternalInput").ap()
    wpT_d = nc.dram_tensor("wpT", [FPC, C], F16, kind="ExternalInput").ap()
    ident_d = nc.dram_tensor("ident", [128, 128], F16, kind="ExternalInput").ap()
    tri_d = nc.dram_tensor("tri", [128, 128], F16, kind="ExternalInput").ap()
    ones_d = nc.dram_tensor("ones", [1, 64], F16, kind="ExternalInput").ap()
    out_d = nc.dram_tensor("out", [T, C], F32, kind="ExternalOutput").ap()

    with tile.TileContext(nc) as tc:
        with tc.tile_pool(name="persist", bufs=1) as persist:
            qt_sb = persist.tile([128, NFB, T], F16, tag="qt")
            kt_sb = persist.tile([128, NFB, T], F16, tag="kt")
            v_sb = persist.tile([128, NKK, HPC, D + 1], F16, tag="v")
            ident = persist.tile([128, 128], F16, tag="ident")
            tri = persist.tile([128, 128], F16, tag="tri")
            ones_col = persist.tile([1, 64], F16, tag="ones")
            nc.sync.dma_start(ident, ident_d)
            nc.sync.dma_start(tri, tri_d)
            nc.sync.dma_start(ones_col, ones_d)
            # ones column of V' (PV matmul then emits softmax denominators)
            nc.gpsimd.memset(v_sb[:, :, :, D:D + 1], 1.0)

            # ---------------- phase 1: QKV projections ----------------
            with tc.tile_pool(name="wqkv", bufs=1) as wqkv, \
                 tc.tile_pool(name="xt", bufs=2) as xtp, \
                 tc.tile_pool(name="ps1", bufs=4, space="PSUM") as ps1:
                wq_sb = wqkv.tile([128, NCC, FPC], F16, tag="wq")
                wk_sb = wqkv.tile([128, NCC, FPC], F16, tag="wk")
                wv_sb = wqkv.tile([128, NCC, FPC], F16, tag="wv")
                wq_r = wqT_d.rearrange("(c p) f -> p c f", p=128)
                wk_r = wkT_d.rearrange("(c p) f -> p c f", p=128)
                wv_r = wvT_d.rearrange("(c p) f -> p c f", p=128)
                xT_r = xT_d.rearrange("(c p) t -> p c t", p=128)

                # first token block's x lands before the weight bulk so the
                # first matmuls can start early
                xt0 = xtp.tile([128, NCC, QB], F16, tag="xt", name="xt0")
                for cc in range(NCC):
                    nc.sync.dma_start(xt0[:, cc, :], xT_r[:, cc, 0:QB])
                for cc in range(NCC):
                    nc.sync.dma_start(wq_sb[:, cc, :], wq_r[:, cc, :])
                    nc.sync.dma_start(wk_sb[:, cc, :], wk_r[:, cc, :])
                    nc.sync.dma_start(wv_sb[:, cc, :], wv_r[:, cc, :])

                # warm the PE clock gate while input DMAs stream in
                warm = ps1.tile([128, 512], F32, tag="ps1", name="warm")
                for i in range(60):
                    nc.tensor.matmul(warm[:, 0:128], ident, ident,
                                     start=True, stop=True,
                                     skip_group_check=True)

                for tb in range(NQB):
                    if tb == 0:
                        xt = xt0
                    else:
                        xt = xtp.tile([128, NCC, QB], F16, tag="xt",
                                      name="xt")
                        for cc in range(NCC):
                            nc.sync.dma_start(
                                xt[:, cc, :],
                                xT_r[:, cc, tb * QB:(tb + 1) * QB])
                    for fb in range(NFB):
                        qps = ps1.tile([128, QB], F32, tag="ps1", name="qps")
                        for cc in range(NCC):
                            nc.tensor.matmul(
                                qps, wq_sb[:, cc, fb * 128:(fb + 1) * 128],
                                xt[:, cc, :],
                                start=(cc == 0), stop=(cc == NCC - 1))
                        nc.vector.tensor_copy(
                            qt_sb[:, fb, tb * QB:(tb + 1) * QB], qps)
                        kps = ps1.tile([128, QB], F32, tag="ps1", name="kps")
                        for cc in range(NCC):
                            nc.tensor.matmul(
                                kps, wk_sb[:, cc, fb * 128:(fb + 1) * 128],
                                xt[:, cc, :],
                                start=(cc == 0), stop=(cc == NCC - 1))
                        nc.vector.tensor_copy(
                            kt_sb[:, fb, tb * QB:(tb + 1) * QB], kps)
                    for tt in range(4):
                        vps = ps1.tile([128, FPC], F32, tag="ps1", name="vps")
                        for cc in range(NCC):
                            nc.tensor.matmul(
                                vps, xt[:, cc, tt * 128:(tt + 1) * 128],
                                wv_sb[:, cc, :],
                                start=(cc == 0), stop=(cc == NCC - 1))
                        nc.vector.tensor_copy(
                            v_sb[:, tb * 4 + tt, :, 0:D],
                            vps.rearrange("p (h d) -> p h d", h=HPC))

            # ------------- phase 2: attention + projection -------------
            with tc.tile_pool(name="const2", bufs=1) as const2, \
                 tc.tile_pool(name="pt", bufs=4) as ptp, \
                 tc.tile_pool(name="yt", bufs=2) as ytp, \
                 tc.tile_pool(name="rcp", bufs=2) as rcpp, \
                 tc.tile_pool(name="sums", bufs=2) as sumsp, \
                 tc.tile_pool(name="outsb", bufs=3) as outp, \
                 tc.tile_pool(name="st", bufs=2, space="PSUM") as stp, \
                 tc.tile_pool(name="pv", bufs=1, space="PSUM") as pvp, \
                 tc.tile_pool(name="bc", bufs=1, space="PSUM") as bcp, \
                 tc.tile_pool(name="prj", bufs=1, space="PSUM") as prjp:
                wp_sb = const2.tile([128, NFB, C], F16, tag="wp")
                nc.sync.dma_start(wp_sb, wpT_d.rearrange("(c p) f -> p c f", p=128))

                for qb in range(NQB):
                    yt = ytp.tile([128, NFB, QB], F16, tag="yt", name="yt")
                    for fb in range(NFB):
                        pv = [pvp.tile([65, QB], F32, tag=f"pv{h2}",
                                       name=f"pv{h2}")
                              for h2 in range(2)]
                        nkk = 4 * qb + 4
                        for kk in range(nkk):
                            dl = kk - 4 * qb
                            j0 = 128 * dl if dl >= 0 else 0
                            st = stp.tile([128, 2, QB], F32, tag="st",
                                          name="st")
                            for h2 in range(2):
                                p0, p1 = 64 * h2, 64 * h2 + 64
                                nc.tensor.matmul(
                                    st[:, h2, j0:QB],
                                    kt_sb[p0:p1, fb, kk * 128:(kk + 1) * 128],
                                    qt_sb[p0:p1, fb, qb * QB + j0:(qb + 1) * QB],
                                    start=True, stop=True,
                                    skip_group_check=True)
                            ptile = ptp.tile([128, 2, QB], F16, tag="pt",
                                             name="ptile")
                            nc.scalar.activation(
                                ptile[:, :, j0:QB], st[:, :, j0:QB], AF.Exp)
                            if dl >= 0:
                                # zero the causally-dead triangle of the
                                # diagonal band on the vector engine (both
                                # heads in one strided op; the 0-step middle
                                # dim re-reads the same mask tile)
                                band = ptile[:, :, j0:j0 + 128]
                                nc.vector.tensor_mul(
                                    band, band,
                                    bass.AP(tri.tensor, tri.offset,
                                            [tri.ap[0], [0, 2], tri.ap[1]]))
                            for h2 in range(2):
                                h = 2 * fb + h2
                                nc.tensor.matmul(
                                    pv[h2][:, j0:QB], v_sb[:, kk, h, :],
                                    ptile[:, h2, j0:QB],
                                    start=(kk == 0), stop=(kk == nkk - 1),
                                    skip_group_check=True)
                        for h2 in range(2):
                            p0, p1 = 64 * h2, 64 * h2 + 64
                            sums = sumsp.tile([1, QB], F16, tag="sums",
                                              name="sums")
                            nc.vector.tensor_copy(sums, pv[h2][D:D + 1, :])
                            bc = bcp.tile([64, QB], F32, tag="bc", name="bc")
                            nc.tensor.matmul(bc, ones_col, sums,
                                             start=True, stop=True)
                            rcp = rcpp.tile([64, QB], F32, tag="rcp",
                                            name="rcp")
                            nc.vector.reciprocal_approx_fast(out=rcp, in_=bc)
                            nc.vector.tensor_mul(yt[p0:p1, fb, :],
                                                 pv[h2][0:D, :], rcp)
                    for tt in range(4):
                        osb = outp.tile([128, C], F32, tag="osb", name="osb")
                        for ofc in range(2):
                            prj = prjp.tile([128, 512], F32, tag="prj",
                                            name="prj")
                            for cc in range(NFB):
                                nc.tensor.matmul(
                                    prj,
                                    yt[:, cc, tt * 128:(tt + 1) * 128],
                                    wp_sb[:, cc, ofc * 512:(ofc + 1) * 512],
                                    start=(cc == 0), stop=(cc == NFB - 1))
                            nc.vector.tensor_copy(
                                osb[:, ofc * 512:(ofc + 1) * 512], prj)
                        nc.sync.dma_start(
                            out_d[qb * QB + tt * 128:qb * QB + (tt + 1) * 128, :],
                            osb)

    nc.compile()
    return nc


def _host_inputs(x, Wk, Wq, Wv, Wp):
    """Build the 8 per-core input maps (host-side slicing/transposes)."""
    ident_np = np.eye(128, dtype=np.float16)
    p = np.arange(128)[:, None]
    jj = np.arange(128)[None, :]
    tri_np = np.where(jj < p, 0.0, 1.0).astype(np.float16)
    ones_np = np.ones((1, 64), dtype=np.float16)

    in_maps = []
    for c in range(N_CORES):
        b, hg = c // 2, c % 2
        fs = slice(hg * FPC, (hg + 1) * FPC)
        in_maps.append({
            "xT": np.ascontiguousarray(x[b].T).astype(np.float16),
            "wqT": np.ascontiguousarray((Wq[fs, :] * 0.125).T).astype(np.float16),
            "wkT": np.ascontiguousarray(Wk[fs, :].T).astype(np.float16),
            "wvT": np.ascontiguousarray(Wv[fs, :].T).astype(np.float16),
            "wpT": np.ascontiguousarray(Wp[:, fs].T).astype(np.float16),
            "ident": ident_np,
            "tri": tri_np,
            "ones": ones_np,
        })
    return in_maps


def kernel(x, Wk, Wq, Wv, Wp, bp, _trace=False):
    x = np.asarray(x, dtype=np.float32)
    Wk = np.asarray(Wk, dtype=np.float32)
    Wq = np.asarray(Wq, dtype=np.float32)
    Wv = np.asarray(Wv, dtype=np.float32)
    Wp = np.asarray(Wp, dtype=np.float32)
    bp = np.asarray(bp, dtype=np.float32)

    if "nc" not in _cached:
        _cached["nc"] = _build_program()
    nc = _cached["nc"]

    in_maps = _host_inputs(x, Wk, Wq, Wv, Wp)
    res = run_bass_kernel_spmd(nc, in_maps, core_ids=list(range(N_CORES)),
                               trace=_trace)
    _cached["last_result"] = res

    out = np.empty((B, T, C), dtype=np.float32)
    for b in range(B):
        out[b] = (res.results[2 * b]["out"].astype(np.float32)
                  + res.results[2 * b + 1]["out"]
                  + bp[None, :])
    return out
